# revision 22
# baseline (speedup 1.0000x reference)
"""AI4DEM DEM step, v5: half-shift pairing + cross-chunk carry on 8 TRN2 cores.

Per half-shift s (62 total), phi_s is computed only on the chunk's own z-planes
(x halo'd, +2 z-planes on the core-top chunk). Contributions:
  +phi(c)          -> identity matmul into PSUM planes [2, dz+2)
  -phi(c+s) gather -> shifted-negated-identity matmul, rhs offset by (sz,sx)
  -phi(c') scatter to c'-s below the chunk -> PSUM carry planes [0, 2),
     saved to SBUF and folded into the next-lower chunk's epilogue
Chunks are processed top-down so each chunk's carry is ready for the next.
PSUM: one 8-bank tile [128, 3*(dz+2)*128] f32, explicitly zeroed per chunk.
"""

import os
import sys

sys.path.insert(0, "/opt/trn_rl_repo")

import numpy as np

N_CORES = 8
Z, Y, X = 256, 128, 128
ZC = Z // N_CORES
DZ = 8
HW_ROW = 136                   # x in [-4, 131]
CHUNKS = []
_z = 0
while _z < ZC:
    CHUNKS.append((_z, min(DZ, ZC - _z)))
    _z += DZ
NCHUNK = int(os.environ.get("DEM_NCHUNK", len(CHUNKS)))

CELL = 0.003
D = CELL
TWO_D = 2.0 * D
KN = 10000.0
_REST = 0.5
_ALPHA = -np.log(_REST) / np.pi
_GAMMA = _ALPHA / np.sqrt(_ALPHA**2 + 1.0)
RHO_P = 1592.0
MASS = 4.0 / 3.0 * 3.1415 * CELL**3 * RHO_P
ETA = 2.0 * _GAMMA * np.sqrt(KN * MASS / 2.0)
ETA_WALL = 2.0 * _GAMMA * np.sqrt(KN * MASS)
DT = 0.0001
EPS = 0.0001
LX, LY, LZ = X * CELL, Y * CELL, Z * CELL
C_F = DT / MASS
R_ED = ETA / KN


def half_shifts():
    out = []
    for sy in (0, -1, 1, -2, 2):
        group = []
        for sz in (0, 1, 2):
            for sx in (-2, -1, 0, 1, 2):
                if sz == 0:
                    if sy == 0 and sx <= 0:
                        continue
                    if sy < 0:
                        continue
                group.append((sz, sx))
        out.append((sy, group))
    return out


HALF = half_shifts()
N_HALF = sum(len(g) for _, g in HALF)
assert N_HALF == 62, N_HALF


def bank_groups(s, e):
    """Split f32-column interval [s, e) at 512-col (2KB bank) boundaries."""
    out = []
    while s < e:
        nxt = min(e, (s // 512 + 1) * 512)
        out.append((s, nxt - s))
        s = nxt
    return out


def build_nc():
    from concourse import bacc, mybir, masks
    from concourse.tile import TileContext

    f32 = mybir.dt.float32
    bf16 = mybir.dt.bfloat16
    A = mybir.AluOpType
    ARS = mybir.ActivationFunctionType.Abs_reciprocal_sqrt

    nc = bacc.Bacc()
    TPL = DZ + 4
    TFREE = TPL * HW_ROW

    pad = nc.declare_dram_parameter(
        "pad", [6, ZC + 8, Y + 4, HW_ROW], f32, isOutput=False
    )
    msk = nc.declare_dram_parameter("msk", [ZC, Y, X], f32, isOutput=False)
    out = nc.declare_dram_parameter("out", [6, ZC, Y, X], f32, isOutput=True)

    with TileContext(nc) as tc:
        with (
            tc.tile_pool(name="const", bufs=1) as constp,
            tc.tile_pool(name="base", bufs=1) as basep,
            tc.tile_pool(name="roll", bufs=1) as rollp,
            tc.tile_pool(name="long", bufs=2) as longp,
            tc.tile_pool(name="scr", bufs=14) as scrp,
            tc.tile_pool(name="carry", bufs=2) as carryp,
            tc.tile_pool(name="psum", bufs=1, space="PSUM") as psump,
        ):
            ident = constp.tile([128, 128], bf16, tag="ident")
            masks.make_identity(nc, ident[:, :])
            negs = {}
            for sy in (-2, -1, 0, 1, 2):
                t = constp.tile([128, 128], bf16, tag=f"neg{sy}", name=f"neg{sy}")
                nc.gpsimd.memset(t[:, :], 0.0)
                nc.gpsimd.affine_select(
                    out=t[:, :], in_=t[:, :], compare_op=A.not_equal,
                    fill=-1.0, base=-sy, pattern=[[-1, 128]], channel_multiplier=1,
                )
                if sy != 0:
                    nc.gpsimd.affine_select(
                        out=t[:, :], in_=t[:, :], compare_op=A.not_equal,
                        fill=-1.0, base=-sy + (128 if sy > 0 else -128),
                        pattern=[[-1, 128]], channel_multiplier=1,
                    )
                negs[sy] = t

            carry_prev = None
            for ck in range(NCHUNK - 1, -1, -1):
                z0, dz = CHUNKS[ck]
                ext = 2 if ck == NCHUNK - 1 else 0
                planes = dz + ext + 2          # input: z0-2 .. z0+dz+ext
                wine = (dz + ext - 1) * HW_ROW + 132
                w0 = 2 * HW_ROW + 2            # window: plane z0, x=-2
                fdo = dz * X
                zstr = (dz + 2) * X            # per-axis stride in acc

                base = {}
                for g in range(2):  # 0: positions, 1: velocities
                    t = basep.tile([128, 3 * TFREE], f32, tag=f"base{g}", name=f"base{g}")
                    t4 = t[:, :].rearrange("p (f z x) -> p f z x", f=3, z=TPL)
                    for j in range(3):
                        nc.sync.dma_start(
                            out=t4[:, j, 0:planes, :],
                            in_=pad[
                                3 * g + j, z0 + 2 : z0 + 2 + planes, 2 : 2 + 128, :
                            ].transpose([1, 0, 2]),
                        )
                    base[g] = t
                mt = constp.tile([128, DZ * X], f32, tag="mask")
                nc.sync.dma_start(
                    out=mt[:, 0:fdo].rearrange("p (z x) -> p z x", z=dz),
                    in_=msk[z0 : z0 + dz, :, :].transpose([1, 0, 2]),
                )

                acc = psump.tile([128, 3 * (DZ + 2) * X], f32, tag="acc")
                nc.vector.memset(acc[:, 0 : 3 * zstr], 0.0)

                for sy, group in HALF:
                    if sy == 0:
                        cur = base
                    else:
                        cur = {}
                        for g in range(2):
                            t = rollp.tile(
                                [128, 3 * TFREE], f32, tag=f"roll{g}", name=f"roll{g}"
                            )
                            t4 = t[:, :].rearrange("p (f z x) -> p f z x", f=3, z=TPL)
                            for j in range(3):
                                nc.sync.dma_start(
                                    out=t4[:, j, 0:planes, :],
                                    in_=pad[
                                        3 * g + j,
                                        z0 + 2 : z0 + 2 + planes,
                                        2 - sy : 130 - sy,
                                        :,
                                    ].transpose([1, 0, 2]),
                                )
                            cur[g] = t
                    for sz, sx in group:
                        no = w0 - sz * HW_ROW - sx
                        cwp = base[0][:, :].rearrange("p (f t) -> p f t", f=3)[
                            :, :, w0 : w0 + wine
                        ]
                        nwp = cur[0][:, :].rearrange("p (f t) -> p f t", f=3)[
                            :, :, no : no + wine
                        ]
                        cwv = base[1][:, :].rearrange("p (f t) -> p f t", f=3)[
                            :, :, w0 : w0 + wine
                        ]
                        nwv = cur[1][:, :].rearrange("p (f t) -> p f t", f=3)[
                            :, :, no : no + wine
                        ]

                        # all 3 position diffs in one op (amortizes op overhead)
                        d_all = longp.tile([128, 3 * wine], bf16, tag="dall")
                        d3 = d_all[:, :].rearrange("p (f t) -> p f t", f=3)
                        nc.vector.tensor_tensor(d3, cwp, nwp, A.subtract)
                        dx = d_all[:, 0:wine]
                        dy = d_all[:, wine : 2 * wine]
                        dzt = d_all[:, 2 * wine : 3 * wine]

                        sq = scrp.tile([128, 3 * wine], bf16, tag="scrb", bufs=4)
                        nc.scalar.square(sq[:, :], d_all[:, :])
                        s12 = scrp.tile([128, wine], bf16, tag="scrs", bufs=6)
                        nc.vector.tensor_tensor(
                            s12[:, :], sq[:, 0:wine], sq[:, wine : 2 * wine], A.add
                        )

                        # velocity diffs on GpSimd (one op): overlaps DVE
                        dv_all = scrp.tile([128, 3 * wine], bf16, tag="scrb", bufs=4)
                        dv3 = dv_all[:, :].rearrange("p (f t) -> p f t", f=3)
                        nc.gpsimd.tensor_tensor(dv3, cwv, nwv, A.subtract)

                        mm = scrp.tile([128, 3 * wine], bf16, tag="scrb", bufs=4)
                        nc.vector.tensor_tensor(mm[:, :], dv_all[:, :], d_all[:, :], A.mult)
                        m3 = scrp.tile([128, wine], bf16, tag="scrs", bufs=6)
                        nc.vector.tensor_tensor(
                            m3[:, :], mm[:, 0:wine], mm[:, wine : 2 * wine], A.add
                        )
                        dvn = scrp.tile([128, wine], bf16, tag="scrs", bufs=6)
                        nc.vector.tensor_tensor(
                            dvn[:, :], m3[:, :], mm[:, 2 * wine : 3 * wine], A.add
                        )

                        # qc = max(z2, eps^2) + s12  (~= max(q, eps^2); differs
                        # only in the deep-eps regime where contributions vanish)
                        qc = scrp.tile([128, wine], bf16, tag="scrs", bufs=6)
                        nc.vector.scalar_tensor_tensor(
                            qc[:, :], sq[:, 2 * wine : 3 * wine], EPS * EPS,
                            s12[:, :], A.max, A.add
                        )
                        invb = longp.tile([128, wine], bf16, tag="invb")
                        nc.scalar.activation(invb[:, :], qc[:, :], ARS)

                        E = scrp.tile([128, wine], bf16, tag="scrs", bufs=6)
                        nc.vector.scalar_tensor_tensor(
                            E[:, :], dvn[:, :], R_ED, qc[:, :], A.mult, A.add
                        )
                        t1 = scrp.tile([128, wine], bf16, tag="scrs", bufs=6)
                        nc.vector.tensor_tensor(t1[:, :], E[:, :], invb[:, :], A.mult)
                        F = scrp.tile([128, wine], bf16, tag="scrs", bufs=6)
                        nc.vector.scalar_tensor_tensor(
                            F[:, :], t1[:, :], TWO_D, invb[:, :], A.subtract, A.mult
                        )
                        Wt = longp.tile([128, wine], bf16, tag="W")
                        nc.vector.tensor_scalar(Wt[:, :], F[:, :], 0.0, None, A.min)

                        PW = (DZ + 3) * HW_ROW
                        P_all = scrp.tile([128, 3 * PW], bf16, tag="scrp", bufs=2)
                        p3 = P_all[:, :].rearrange("p (f t) -> p f t", f=3)[
                            :, :, 0:wine
                        ]
                        wb = Wt[:, :].unsqueeze(1).to_broadcast([128, 3, wine])
                        nc.vector.tensor_tensor(p3, wb, d3, A.mult)
                        for a in range(3):
                            P = P_all[:, a * PW : (a + 1) * PW]
                            Ab = a * zstr
                            # center: +phi(c), acc planes [2, dz+2)
                            for o, n in bank_groups(Ab + 2 * X, Ab + (dz + 2) * X):
                                cz0 = (o - Ab) // X - 2
                                zp = n // X
                                rhs = P[
                                    :, cz0 * HW_ROW + 2 : (cz0 + zp) * HW_ROW + 2
                                ].rearrange("p (z x) -> p z x", z=zp)[:, :, 0:128]
                                ov = acc[:, o : o + n].rearrange(
                                    "p (z x) -> p z x", z=zp
                                )
                                nc.tensor.matmul(
                                    ov, ident[:, :], rhs,
                                    start=False, stop=False, skip_group_check=True,
                                )
                            # minus-gather: -phi(c+s), c_z in [0, dzg)
                            dzg = dz if ext >= sz else dz - sz
                            for o, n in bank_groups(Ab + 2 * X, Ab + (2 + dzg) * X):
                                cz0 = (o - Ab) // X - 2
                                zp = n // X
                                st = (cz0 + sz) * HW_ROW + sx + 2
                                rhs = P[:, st : st + zp * HW_ROW].rearrange(
                                    "p (z x) -> p z x", z=zp
                                )[:, :, 0:128]
                                ov = acc[:, o : o + n].rearrange(
                                    "p (z x) -> p z x", z=zp
                                )
                                nc.tensor.matmul(
                                    ov, negs[sy][:, :], rhs,
                                    start=False, stop=False, skip_group_check=True,
                                )
                            # minus-carry: -phi(c'), c'_z in [0, sz) -> planes [2-sz, 2)
                            if sz > 0:
                                for o, n in bank_groups(
                                    Ab + (2 - sz) * X, Ab + 2 * X
                                ):
                                    pz0 = (o - Ab) // X
                                    zp = n // X
                                    st = (pz0 - 2 + sz) * HW_ROW + sx + 2
                                    rhs = P[:, st : st + zp * HW_ROW].rearrange(
                                        "p (z x) -> p z x", z=zp
                                    )[:, :, 0:128]
                                    ov = acc[:, o : o + n].rearrange(
                                        "p (z x) -> p z x", z=zp
                                    )
                                    nc.tensor.matmul(
                                        ov, negs[sy][:, :], rhs,
                                        start=False, stop=False,
                                        skip_group_check=True,
                                    )

                # save carry planes for the next-lower chunk
                carry = None
                if ck > 0:
                    carry = carryp.tile([128, 3 * 2 * X], f32, tag="carry")
                    for a in range(3):
                        nc.vector.tensor_copy(
                            carry[:, a * 2 * X : (a + 1) * 2 * X],
                            acc[:, a * zstr : a * zstr + 2 * X],
                        )

                # ---- epilogue ----
                c3 = [
                    base[f // 3][:, :]
                    .rearrange("p (f z x) -> p f z x", f=3, z=TPL)[
                        :, f % 3, 2 : 2 + dz, 4 : 4 + 128
                    ]
                    for f in range(6)
                ]
                m3v = mt[:, 0:fdo].rearrange("p (z x) -> p z x", z=dz)
                wall_cfg = [
                    (0, 3, D, LX - TWO_D, D),
                    (1, 4, D, LY - TWO_D, D),
                    (2, 5, 3.0 * D, LZ - TWO_D, 3.0 * D),
                ]
                for a, (pi, vi, lo_thr, hi_thr, lo_base) in enumerate(wall_cfg):
                    Xc, Vc = c3[pi], c3[vi]
                    wv = acc[:, a * zstr + 2 * X : a * zstr + (2 + dz) * X].rearrange(
                        "p (z x) -> p z x", z=dz
                    )
                    il = scrp.tile([128, DZ * X], f32, tag="scr", bufs=4)
                    il3 = il[:, 0:fdo].rearrange("p (z x) -> p z x", z=dz)
                    t_a = scrp.tile([128, DZ * X], f32, tag="scr", bufs=4)
                    t_a3 = t_a[:, 0:fdo].rearrange("p (z x) -> p z x", z=dz)
                    nc.vector.tensor_scalar(il3, Xc, lo_thr, None, A.is_lt)
                    nc.vector.tensor_scalar(t_a3, Xc, 0.0, None, A.not_equal)
                    nc.vector.tensor_tensor(il3, il3, t_a3, A.mult)
                    ir = scrp.tile([128, DZ * X], f32, tag="scr", bufs=4)
                    ir3 = ir[:, 0:fdo].rearrange("p (z x) -> p z x", z=dz)
                    nc.vector.tensor_scalar(ir3, Xc, hi_thr, None, A.is_gt)
                    u1 = scrp.tile([128, DZ * X], f32, tag="scr", bufs=4)
                    u13 = u1[:, 0:fdo].rearrange("p (z x) -> p z x", z=dz)
                    nc.vector.tensor_scalar(u13, Xc, lo_base, -1.0, A.subtract, A.mult)
                    nc.vector.tensor_tensor(u13, u13, il3, A.mult)
                    u2 = scrp.tile([128, DZ * X], f32, tag="scr", bufs=4)
                    u23 = u2[:, 0:fdo].rearrange("p (z x) -> p z x", z=dz)
                    nc.vector.tensor_scalar(u23, Xc, hi_thr, None, A.subtract)
                    nc.vector.tensor_tensor(u23, u23, ir3, A.mult)
                    nc.vector.tensor_tensor(u13, u13, u23, A.subtract)
                    nc.vector.tensor_tensor(u13, u13, wv, A.subtract)
                    if carry_prev is not None:
                        top = u1[:, (dz - 2) * X : dz * X].rearrange(
                            "p (z x) -> p z x", z=2
                        )
                        cp = carry_prev[:, a * 2 * X : (a + 1) * 2 * X].rearrange(
                            "p (z x) -> p z x", z=2
                        )
                        nc.vector.tensor_tensor(top, top, cp, A.subtract)
                    nc.vector.tensor_tensor(ir3, il3, ir3, A.add)
                    nc.vector.scalar_tensor_tensor(
                        ir3, Vc, -C_F * ETA_WALL, ir3, A.mult, A.mult
                    )
                    nc.vector.scalar_tensor_tensor(
                        u13, u13, C_F * KN, ir3, A.mult, A.add
                    )
                    if a == 2:
                        nc.vector.tensor_scalar(u13, u13, DT * -9.8, None, A.add)
                    nc.vector.tensor_tensor(u13, u13, m3v, A.mult)
                    vn = scrp.tile([128, DZ * X], f32, tag="scr", bufs=4)
                    vn3 = vn[:, 0:fdo].rearrange("p (z x) -> p z x", z=dz)
                    nc.vector.tensor_tensor(vn3, Vc, u13, A.add)
                    xn = scrp.tile([128, DZ * X], f32, tag="scr", bufs=4)
                    xn3 = xn[:, 0:fdo].rearrange("p (z x) -> p z x", z=dz)
                    nc.vector.scalar_tensor_tensor(xn3, vn3, DT, Xc, A.mult, A.add)
                    nc.sync.dma_start(
                        out=out[a, z0 : z0 + dz, :, :].transpose([1, 0, 2]), in_=xn3
                    )
                    nc.sync.dma_start(
                        out=out[3 + a, z0 : z0 + dz, :, :].transpose([1, 0, 2]),
                        in_=vn3,
                    )
                carry_prev = carry
    nc.compile()
    return nc


_NC = None


def _get_nc():
    global _NC
    if _NC is None:
        _NC = build_nc()
    return _NC


def shard_inputs(x_grid, y_grid, z_grid, vx_grid, vy_grid, vz_grid, mask):
    F = np.stack(
        [
            np.asarray(a, dtype=np.float32).reshape(Z, Y, X)
            for a in (x_grid, y_grid, z_grid, vx_grid, vy_grid, vz_grid)
        ]
    )
    Fp = np.pad(F, ((0, 0), (4, 4), (2, 2), (4, 4)), mode="wrap")
    mk = np.asarray(mask, dtype=np.float32).reshape(Z, Y, X)
    in_maps = []
    for c in range(N_CORES):
        in_maps.append(
            {
                "pad": np.ascontiguousarray(Fp[:, c * ZC : c * ZC + ZC + 8]),
                "msk": np.ascontiguousarray(mk[c * ZC : c * ZC + ZC]),
            }
        )
    return in_maps


def assemble(results):
    full = np.empty((6, 1, 1, Z, Y, X), dtype=np.float32)
    for c in range(N_CORES):
        full[:, 0, 0, c * ZC : (c + 1) * ZC] = results[c]["out"]
    return full


def kernel(**inputs):
    from concourse.bass_utils import run_bass_kernel_spmd

    nc = _get_nc()
    in_maps = shard_inputs(**inputs)
    res = run_bass_kernel_spmd(nc, in_maps, list(range(N_CORES)))
    return assemble(res.results)
